# revision 57
# baseline (speedup 1.0000x reference)
"""Trainium2 Bass kernel for the stacked-LSTM model (nn_Model2_16904991277618).

Model: LSTM-A(64->40, return_sequences) -> LSTM-B(40->40, last) over T=1024,
plus a small dense tail on `feat`, concat, 3 dense layers -> sigmoid [B,1].

Strategy: data-parallel over batch (B=512 -> 64 rows/core on 8 cores), and
within each core TWO independent half-batch groups (BCg=32) whose serial
scan chains interleave on the engines, halving per-op payloads.

Per step the A and B cells ride SHARED instructions (B lags A by one step):
  zz PSUM [128, 4*BCg], col-blocks [Afo | Bfo | Aig | Big]:
     f @ rows 0:40, o @ rows 64:104 (fo blocks); i @ 0:40, 2g @ 64:104 (ig).
  ONE sigmoid covers all gates of both cells; tanh(g) = 2*sig(2g)-1 via a
  DVE affine (g-weights pre-scaled by 2).
  h-ring H [105, R*BCg] bf16: rows 0:40 = hA(t), rows 64:104 = hB(t-1),
  row 104 = ones (carries B's bias). B's whole input projection is then a
  SINGLE matmul per gate-pair: lhsT [105,128] = [WbK; 0; WbR; bb] @ H-slot.
  A's bias ba rides the x ones-row. No separate bias matmuls, no h copies:
  the two h-mults write straight into the ring the matmuls read.
"""

import functools
import os
import sys

import numpy as np

for _p in ("/opt/trn_rl_repo", "/root/.axon_site/_ro/trn_rl_repo"):
    if os.path.isdir(_p) and _p not in sys.path:
        sys.path.insert(0, _p)

import ml_dtypes  # noqa: E402

import concourse.bass as bass  # noqa: E402
import concourse.bacc as bacc  # noqa: E402
import concourse.mybir as mybir  # noqa: E402
import concourse.tile as tile  # noqa: E402
from concourse.bass_utils import run_bass_kernel_spmd  # noqa: E402

F32 = mybir.dt.float32
BF16 = mybir.dt.bfloat16
AF = mybir.ActivationFunctionType
OP = mybir.AluOpType

NCORES = 8
H = 40
D = 10
F = 64
G = 2          # independent half-batch groups per core
RING = 4       # h-ring slots

# gate column ranges in the reference [*, 4H] weight matrices
_I, _Fg, _G, _O = slice(0, 40), slice(40, 80), slice(80, 120), slice(120, 160)


def _bf(x):
    return np.ascontiguousarray(x, dtype=ml_dtypes.bfloat16)


def _f32c(x):
    return np.ascontiguousarray(x, dtype=np.float32)


def _pack2(w, s0, s1, s1_scale=1.0):
    """[k, 4H] -> [k, 128]: gate s0 at cols 0:40, s1 (scaled) at 64:104."""
    w = np.asarray(w, np.float32)
    out = np.zeros((w.shape[0], 128), np.float32)
    out[:, 0:40] = w[:, s0]
    out[:, 64:104] = w[:, s1] * s1_scale
    return out


def _pack_hi(w, s0, scale=1.0):
    """[k, 4H] -> [k, 128]: single gate (scaled) at cols 64:104."""
    w = np.asarray(w, np.float32)
    out = np.zeros((w.shape[0], 128), np.float32)
    out[:, 64:104] = w[:, s0] * scale
    return out


def _pack_lo(w, s0, scale=1.0):
    """[k, 4H] -> [k, 128]: single gate (scaled) at cols 0:40."""
    w = np.asarray(w, np.float32)
    out = np.zeros((w.shape[0], 128), np.float32)
    out[:, 0:40] = w[:, s0] * scale
    return out


def _build_program(T, BC):
    BCg = BC // G
    CHUNK_T = min(T, 128)
    n_chunks = T // CHUNK_T
    assert n_chunks * CHUNK_T == T
    B1, B2, B3, B4 = BCg, 2 * BCg, 3 * BCg, 4 * BCg

    nc = bacc.Bacc("TRN2", debug=False, target_bir_lowering=False,
                   num_devices=NCORES)

    def din(name, shape, dt):
        return nc.dram_tensor(name, list(shape), dt, kind="ExternalInput").ap()

    xt = [din(f"xt{g}", (n_chunks, F + 1, CHUNK_T * BCg), BF16)
          for g in range(G)]
    featT = din("featT", (F, BC), BF16)
    d_in = {}
    for nm in ("f", "o", "i", "g"):
        d_in[f"wa_x_{nm}"] = din(f"wa_x_{nm}", (F + 1, 128), BF16)
        d_in[f"wa_h_{nm}"] = din(f"wa_h_{nm}", (H, 128), BF16)
        d_in[f"wb_{nm}"] = din(f"wb_{nm}", (2 * H + 25, 128), BF16)
    d_in.update({
        "wg": din("wg", (F, D), BF16),
        "wh": din("wh", (D, D), BF16),
        "wc": din("wc", (74, 2 * D), BF16),
        "wd": din("wd", (2 * D, D), BF16),
        "wo": din("wo", (D, 1), BF16),
        "bg": din("bg", (D, 1), F32),
        "bh": din("bh", (D, 1), F32),
        "bc2": din("bc2", (2 * D, 1), F32),
        "bd": din("bd", (D, 1), F32),
        "bo": din("bo", (1, 1), F32),
    })

    out_dram = nc.dram_tensor("out", [1, BC], F32, kind="ExternalOutput").ap()

    from contextlib import ExitStack

    with tile.TileContext(nc) as tc:
        with ExitStack() as ctx:
            wpool = ctx.enter_context(tc.tile_pool(name="w", bufs=1))
            xpool = ctx.enter_context(tc.tile_pool(name="x", bufs=1))
            gpool = ctx.enter_context(tc.tile_pool(name="g", bufs=3))
            tpool = ctx.enter_context(tc.tile_pool(name="t", bufs=3))
            cpool = ctx.enter_context(tc.tile_pool(name="c", bufs=3))
            spool = ctx.enter_context(tc.tile_pool(name="s", bufs=1))
            psum = ctx.enter_context(tc.tile_pool(name="ps", bufs=3,
                                                  space="PSUM"))

            W = {}
            for nm, src in d_in.items():
                t = wpool.tile(list(src.shape), src.dtype, name=f"w_{nm}")
                nc.sync.dma_start(t[:], src[:])
                W[nm] = t
            ftile = wpool.tile([F, BC], BF16, name="w_featT")
            nc.sync.dma_start(ftile[:], featT[:])

            # x chunks per group
            xch = [[None] * n_chunks for _ in range(G)]
            for ci in range(n_chunks):
                for g in range(G):
                    xc = xpool.tile([F + 1, CHUNK_T * BCg], BF16,
                                    name=f"xc{g}_{ci}", tag=f"xc{g}_{ci}")
                    nc.sync.dma_start(xc[:], xt[g][ci])
                    xch[g][ci] = xc

            # h-ring per group: rows 0:40 hA(t), 64:104 hB(t-1), 104 ones
            Hr = []
            for g in range(G):
                hr = wpool.tile([2 * H + 25, RING * BCg], BF16, name=f"H{g}")
                nc.gpsimd.memset(hr[:], 0.0)
                # ones row lives at partition 104; ops must start at a
                # multiple of 32, so write 96:105 then re-zero 96:104
                nc.gpsimd.memset(hr[96:2 * H + 25, :], 1.0)
                nc.gpsimd.memset(hr[96:2 * H + 24, :], 0.0)
                Hr.append(hr)

            # initial c (cols [cA | cB]) per group
            cprev = []
            for g in range(G):
                c0 = cpool.tile([104, BCg], BF16, name=f"c0_{g}",
                                tag=f"c{g}")
                nc.gpsimd.memset(c0[:], 0.0)
                cprev.append(c0)

            state = [dict(c=cprev[g], zz=None, gp=None, cnew=None, tc=None,
                          tg_ins=None)
                     for g in range(G)]

            from concourse.tile import add_dep_helper

            def emit_mms(g, it):
                """All matmuls for iteration it of group g into a fresh zz.

                Cell B processes timestep it-2, so its matmuls read only old
                ring slots and run early; only A's h-matmuls sit on the
                critical chain.  A cross-group false dep on the sibling's tg
                pins the two groups half a lap apart (no FIFO bunching).
                """
                # per-quantity col blocks [f|o|i|g], A rows 0:40 (lhsT cols
                # 0:40), B rows 64:104 (lhsT cols 64:104), accumulated in one
                # PSUM group per block: x(start) -> A-h -> B(stop)
                zz = psum.tile([128, 4 * BCg], F32, name=f"zz{g}_{it % 2}",
                               tag=f"zz{g}")
                hr = Hr[g]
                gates = ("f", "o", "i", "g")
                if it < T:
                    ci, tl = divmod(it, CHUNK_T)
                    xr = xch[g][ci][:, tl * BCg:(tl + 1) * BCg]
                    for j, nm in enumerate(gates):
                        nc.tensor.matmul(zz[:, j * B1:(j + 1) * B1],
                                         W[f"wa_x_{nm}"][:], xr,
                                         start=True, stop=(it == 0))
                if 0 < it < T:
                    sp = ((it - 1) % RING) * BCg
                    ha = hr[0:H, sp:sp + BCg]
                    for j, nm in enumerate(gates):
                        i_h = nc.tensor.matmul(zz[:, j * B1:(j + 1) * B1],
                                               W[f"wa_h_{nm}"][:], ha,
                                               start=False, stop=(it < 2))
                        if g == 1 and j == 0:
                            # stagger floor: group 1's chain trails group
                            # 0's sigmoid so the laps never FIFO-bunch
                            sib = state[0].get("sig_ins")
                            if sib is not None:
                                add_dep_helper(i_h.ins, sib.ins, False, "st")
                if it >= 2:
                    sp2 = ((it - 2) % RING) * BCg
                    hab = hr[0:2 * H + 25, sp2:sp2 + BCg]
                    for j, nm in enumerate(gates):
                        nc.tensor.matmul(zz[:, j * B1:(j + 1) * B1],
                                         W[f"wb_{nm}"][:], hab,
                                         start=(it >= T), stop=True)
                state[g]["zz"] = zz

            def emit_sig(g, it):
                zz = state[g]["zz"]
                gp = gpool.tile([104, 4 * BCg], BF16, name=f"gp{g}",
                                tag=f"gp{g}")
                lo, hi = 0, 104
                if it < 2:
                    lo, hi = 0, 40      # A rows only
                elif it >= T:
                    lo, hi = 64, 104    # B rows only
                i_s = nc.scalar.activation(gp[lo:hi, :], zz[lo:hi, 0:B4],
                                           AF.Sigmoid)
                state[g]["gp"] = gp
                state[g]["sig_ins"] = i_s
                state[g]["rows"] = (lo, hi)

            def emit_dve(g, it):
                """p, mh, c_new over [f|o|i|g] blocks; A rows 0:40, B rows
                64:104 (rows 40:64 are zero junk).  Boundary narrowing is a
                partition slice."""
                gp = state[g]["gp"]
                c_old = state[g]["c"]
                lo, hi = state[g]["rows"]
                p = tpool.tile([104, BCg], BF16, name=f"p{g}", tag=f"p{g}")
                nc.gpsimd.tensor_tensor(p[lo:hi, :], gp[lo:hi, 0:B1],
                                        c_old[lo:hi, :], OP.mult)
                # mh = (sig(2g) - 0.5) * sig(i) = i*tanh(g)/2 ; c/2 = mh + p
                m = tpool.tile([104, BCg], BF16, name=f"m{g}", tag=f"m{g}")
                nc.vector.scalar_tensor_tensor(m[lo:hi, :],
                                               gp[lo:hi, B3:B4], 0.5,
                                               gp[lo:hi, B2:B3],
                                               OP.subtract, OP.mult)
                # state is c/2: mh = i*tanh(g)/2, p = f*(c/2); the missing
                # factor 2 rides the tanh's scale parameter
                cn = cpool.tile([104, BCg], BF16, name=f"c{g}", tag=f"c{g}")
                nc.vector.tensor_tensor(cn[lo:hi, :], m[lo:hi, :],
                                        p[lo:hi, :], OP.add)
                if it < 2:
                    nc.gpsimd.memset(cn[64:104, :], 0.0)
                state[g]["cnew"] = (cn, lo, hi)

            def emit_tail(g, it):
                """tanh + the two h-mults into the ring."""
                gp = state[g]["gp"]
                cn, lo, hi = state[g]["cnew"]
                tcp = gpool.tile([104, BCg], BF16, name=f"tc{g}",
                                 tag=f"tc{g}")
                nc.scalar.activation(tcp[lo:hi, :], cn[lo:hi, :], AF.Tanh,
                                     scale=2.0)
                hr = Hr[g]
                if it < T:
                    sc = (it % RING) * BCg
                    nc.vector.tensor_tensor(hr[0:H, sc:sc + BCg],
                                            gp[0:40, B1:B2],
                                            tcp[0:40, :], OP.mult)
                if it >= 2:  # hB(it-2) lands in slot it-1 next to hA(it-1)
                    sb = ((it - 1) % RING) * BCg
                    nc.vector.tensor_tensor(hr[64:104, sb:sb + BCg],
                                            gp[64:104, B1:B2],
                                            tcp[64:104, :], OP.mult)
                state[g]["c"] = cn

            # Software-pipelined emission: group 1 runs ~half a step behind
            # group 0, enforced by queue order.
            for it in range(T + 2):
                if it > 0:
                    emit_tail(1, it - 1)
                emit_mms(0, it)
                emit_sig(0, it)
                emit_dve(0, it)
                emit_mms(1, it)
                emit_sig(1, it)
                emit_dve(1, it)
                emit_tail(0, it)
            emit_tail(1, T + 1)

            # ---- dense tail ----
            fslot = (T % RING) * BCg
            zcat = spool.tile([74, BC], BF16, name="zcat")
            nc.gpsimd.memset(zcat[:], 0.0)
            for g in range(G):
                nc.vector.tensor_copy(zcat[0:40, g * BCg:(g + 1) * BCg],
                                      Hr[g][64:104, fslot:fslot + BCg])

            ps1 = psum.tile([D, BC], F32, name="ps1", tag="zz0")
            nc.tensor.matmul(ps1[:], W["wg"][:], ftile[:],
                             start=True, stop=True)
            y1 = spool.tile([D, BC], BF16, name="y1")
            nc.scalar.activation(y1[:], ps1[:], AF.Tanh, bias=W["bg"][:])

            ps2 = psum.tile([D, BC], F32, name="ps2", tag="zz1")
            nc.tensor.matmul(ps2[:], W["wh"][:], y1[:], start=True, stop=True)
            nc.scalar.activation(zcat[64:74, :], ps2[:], AF.Tanh,
                                 bias=W["bh"][:])

            ps3 = psum.tile([2 * D, BC], F32, name="ps3", tag="zz0")
            nc.tensor.matmul(ps3[:], W["wc"][:], zcat[:], start=True,
                             stop=True)
            c1 = spool.tile([2 * D, BC], BF16, name="c1")
            nc.scalar.activation(c1[:], ps3[:], AF.Relu, bias=W["bc2"][:])

            ps4 = psum.tile([D, BC], F32, name="ps4", tag="zz1")
            nc.tensor.matmul(ps4[:], W["wd"][:], c1[:], start=True, stop=True)
            d1 = spool.tile([D, BC], BF16, name="d1")
            nc.scalar.activation(d1[:], ps4[:], AF.Relu, bias=W["bd"][:])

            ps5 = psum.tile([1, BC], F32, name="ps5", tag="zz0")
            nc.tensor.matmul(ps5[:], W["wo"][:], d1[:], start=True, stop=True)
            osb = spool.tile([1, BC], F32, name="osb")
            nc.scalar.activation(osb[:], ps5[:], AF.Sigmoid, bias=W["bo"][:])

            nc.sync.dma_start(out_dram[:], osb[:])

    nc.compile()
    return nc


@functools.lru_cache(maxsize=2)
def _program(T, BC):
    return _build_program(T, BC)


def _prep_shared(Wa_k, Wa_r, ba, Wb_k, Wb_r, bb, Wg, bg, Wh, bh, Wc, bc, Wd,
                 bd, Wo, bo):
    ba = np.asarray(ba, np.float32)[None, :]
    bb = np.asarray(bb, np.float32)[None, :]

    wc_re = np.zeros((74, 2 * D), np.float32)
    wc_re[0:40] = np.asarray(Wc, np.float32)[0:40]
    wc_re[64:74] = np.asarray(Wc, np.float32)[40:50]
    ret = {}
    for nm, sl, sc in (("f", _Fg, 1.0), ("o", _O, 1.0), ("i", _I, 1.0),
                       ("g", _G, 2.0)):
        ret[f"wa_x_{nm}"] = _bf(np.concatenate([_pack_lo(Wa_k, sl, sc),
                                                _pack_lo(ba, sl, sc)]))
        ret[f"wa_h_{nm}"] = _bf(_pack_lo(Wa_r, sl, sc))
        wb = np.zeros((2 * H + 25, 128), np.float32)
        wb[0:H] = _pack_hi(Wb_k, sl, sc)
        wb[64:104] = _pack_hi(Wb_r, sl, sc)
        wb[104:105] = _pack_hi(bb, sl, sc)
        ret[f"wb_{nm}"] = _bf(wb)
    ret.update({
        "wg": _bf(Wg), "wh": _bf(Wh), "wc": _bf(wc_re), "wd": _bf(Wd),
        "wo": _bf(Wo),
        "bg": _f32c(np.asarray(bg)[:, None]),
        "bh": _f32c(np.asarray(bh)[:, None]),
        "bc2": _f32c(np.asarray(bc)[:, None]),
        "bd": _f32c(np.asarray(bd)[:, None]),
        "bo": _f32c(np.asarray(bo)[:, None]),
    })
    return ret


def _prep_seq(seq, T, BCg, CHUNK_T):
    n_chunks = T // CHUNK_T
    arr = np.asarray(seq, np.float32).reshape(NCORES, G, BCg, n_chunks,
                                              CHUNK_T, F)
    arr = arr.transpose(0, 1, 3, 5, 4, 2)  # [c, g, ci, F, tl, j]
    arr = arr.reshape(NCORES, G, n_chunks, F, CHUNK_T * BCg)
    ones = np.ones((NCORES, G, n_chunks, 1, CHUNK_T * BCg), np.float32)
    return _bf(np.concatenate([arr, ones], axis=3))


def kernel(seq, feat, Wa_k, Wa_r, ba, Wb_k, Wb_r, bb, Wg, bg, Wh, bh, Wc, bc,
           Wd, bd, Wo, bo, _trace=False):
    seq = np.asarray(seq)
    feat = np.asarray(feat)
    B, T, _ = seq.shape
    assert B % (NCORES * G) == 0
    BC = B // NCORES
    BCg = BC // G
    CHUNK_T = min(T, 128)

    nc = _program(T, BC)

    shared = _prep_shared(Wa_k, Wa_r, ba, Wb_k, Wb_r, bb, Wg, bg, Wh, bh, Wc,
                          bc, Wd, bd, Wo, bo)
    xt = _prep_seq(seq, T, BCg, CHUNK_T)
    featc = np.asarray(feat, np.float32).reshape(NCORES, BC, F)

    in_maps = []
    for c in range(NCORES):
        m = dict(shared)
        for g in range(G):
            m[f"xt{g}"] = xt[c, g]
        m["featT"] = _bf(featc[c].T)
        in_maps.append(m)

    res = run_bass_kernel_spmd(nc, in_maps, core_ids=list(range(NCORES)),
                               trace=_trace)
    out = np.concatenate([res.results[c]["out"][0] for c in range(NCORES)])
    out = out.astype(np.float32).reshape(B, 1)
    if _trace:
        kernel.last_results = res
    return out
